# revision 8
# baseline (speedup 1.0000x reference)
"""Bahdanau-attention pooling kernel for TRN2, data-parallel over 8 NeuronCores.

Reference computation (per batch b):
    h   = tanh(enc @ W1enc.T + hid @ W1hid.T + b1)    [S, K]   (K = D = 512)
    e   = h @ w2                                       [S]
    a   = softmax(e)                                   [S]
    ctx = a @ enc                                      [D]

Distribution: batch dim (32) sharded 4-per-core across 8 cores; replicated
weights, no collectives.

v3 design (per core, single pass over the encoder stream):
  - scores: enc ships as e4m3 [d, s] tiles; W1enc pre-scaled x16 and
    quantized e4m3; h-matmuls run in DoubleRow fp8 mode (256-deep
    contraction per pass); the 1/16 rescale is folded into the tanh scale.
    tanh is j-pair fused (one ACT per kc chunk covers both tiles of a pair).
    e-matmul stays bf16 (h8/w2 quantization there costs too much accuracy).
  - context: runs on the PE, not DVE. enc also ships as e3m4 [s, d] tiles
    (natural layout, error-diffusion rounded along s so quantization noise
    cancels in the softmax average), with s interleaved as s = 4p + c so a
    plain [1,512]->[128,4] DMA produces the p-column tiles. Each tile then
    adds 4 rank-1 matmuls (p-col x enc-rows), column-tiled to PSUM partitions
    {0,32,64,96}, accumulated across the whole batch in one PSUM bank.
  - exp's accum_out lands z in per-tile slots, reduced once per batch.
  - softmax is max-free: |e| <= ||w2||_1 ~ 11, exp never overflows fp32.
  - ctx matmuls are emitted one pair late (software pipelining) so the PE
    never head-of-line blocks on the exp -> pT DMA chain.
"""

import numpy as np

B, S, D = 32, 4096, 512
N_CORES = 8
B_LOC = B // N_CORES
T = 512          # s-tile size
KC = D // 128    # 4 k-chunks
DC = D // 128    # 4 d-chunks
W_SCALE = 16.0   # host pre-scale on W1enc before e4m3 quantization


def build_nc(b_loc=B_LOC, s_len=S, t=T):
    import concourse.bass as bass
    import concourse.mybir as mybir
    import concourse.tile as tile

    fp32 = mybir.dt.float32
    f32r = mybir.dt.float32r
    bf16 = mybir.dt.bfloat16
    f8e4 = mybir.dt.float8e4
    f8e3 = mybir.dt.float8e3
    AF = mybir.ActivationFunctionType
    Alu = mybir.AluOpType
    DR = mybir.MatmulPerfMode.DoubleRow

    nc = bass.Bass()

    n_tiles = s_len // t

    enc8_ext = nc.declare_dram_parameter(
        "enc8", [b_loc, n_tiles, 128, DC, t], f8e4, isOutput=False)
    encq3_ext = nc.declare_dram_parameter(
        "encq3", [b_loc, n_tiles, 128, 4, D], f8e3, isOutput=False)
    hid_ext = nc.declare_dram_parameter("hid", [b_loc, D], bf16, isOutput=False)
    w1et8_ext = nc.declare_dram_parameter(
        "w1et8", [KC, 128, DC, 128], f8e4, isOutput=False)
    w1ht_ext = nc.declare_dram_parameter(
        "w1ht", [KC, 128, DC, 128], bf16, isOutput=False)
    b1_ext = nc.declare_dram_parameter("b1", [D], fp32, isOutput=False)
    w28_ext = nc.declare_dram_parameter("w28", [KC, 128], bf16, isOutput=False)
    out_ext = nc.declare_dram_parameter("out", [b_loc, D], fp32, isOutput=True)

    with tile.TileContext(nc) as tc:
        with (
            tc.tile_pool(name="singles", bufs=1) as singles,
            tc.tile_pool(name="enc8_pool", bufs=4) as enc8_pool,
            tc.tile_pool(name="enc3_pool", bufs=4) as enc3_pool,
            tc.tile_pool(name="h8_pool", bufs=2) as h8_pool,
            tc.tile_pool(name="p_pool", bufs=4) as p_pool,
            tc.tile_pool(name="pt_pool", bufs=4) as pt_pool,
            tc.tile_pool(name="tiny", bufs=4) as tiny,
            tc.tile_pool(name="ps_h", bufs=2, space=bass.MemorySpace.PSUM) as ps_h,
            tc.tile_pool(name="ps_e", bufs=2, space=bass.MemorySpace.PSUM) as ps_e,
            tc.tile_pool(name="ps_c", bufs=1, space=bass.MemorySpace.PSUM) as ps_c,
        ):
            # ---- persistent tiles (small gpsimd loads + per-kc weight DMAs) --
            hid_cols = singles.tile([128, b_loc, DC], bf16)  # [p(d), b, d-chunk]
            nc.gpsimd.dma_start(
                out=hid_cols, in_=hid_ext.rearrange("b (c p) -> p b c", p=128)
            )
            b1_col = singles.tile([128, KC], fp32)
            nc.gpsimd.dma_start(out=b1_col, in_=b1_ext.rearrange("(c p) -> p c", p=128))
            w2_col = singles.tile([128, KC], bf16)
            nc.gpsimd.dma_start(out=w2_col, in_=w28_ext.rearrange("c p -> p c"))

            w1et8_sb = singles.tile([128, KC, DC, 128], f8e4)
            w1ht_sb = singles.tile([128, KC, DC, 128], bf16)
            for kc in range(KC):
                nc.scalar.dma_start(out=w1et8_sb[:, kc, :, :], in_=w1et8_ext[kc])
                nc.gpsimd.dma_start(out=w1ht_sb[:, kc, :, :], in_=w1ht_ext[kc])

            # mask column: 1.0 at partitions {0,32,64,96} picks the 4 ctx rows
            ones4 = singles.tile([128, 1], fp32)
            nc.vector.memset(ones4, 0.0)
            for c in range(4):
                nc.vector.memset(ones4[32 * c:32 * c + 1, :], 1.0)

            r_sb = singles.tile([128, KC, b_loc], fp32)   # [p(k), k-chunk, b]
            z_parts = singles.tile([1, n_tiles], fp32)

            ctx_ps = ps_c.tile([128, t], fp32, tag="ctx")
            nc.vector.memset(ctx_ps, 0.0)

            # ---- r = W1hid @ hid + b1  (bf16, all batches at once) ---------
            for kc in range(KC):
                r_ps = ps_e.tile([128, b_loc], fp32, tag="e")
                for dc in range(DC):
                    nc.tensor.matmul(
                        r_ps,
                        w1ht_sb[:, kc, dc, :],
                        hid_cols[:, :, dc],
                        start=(dc == 0),
                        stop=(dc == DC - 1),
                    )
                nc.vector.tensor_scalar_add(
                    out=r_sb[:, kc, :], in0=r_ps, scalar1=b1_col[:, kc:kc + 1]
                )

            # ---- main loop ------------------------------------------------
            for b in range(b_loc):
                pending_ctx = []  # (pT, enc3_tile, ti) emitted one pair late

                def flush_ctx():
                    while pending_ctx:
                        pT, e3t, ti = pending_ctx.pop(0)
                        for c in range(4):
                            nc.tensor.matmul(
                                ctx_ps[32 * c:32 * c + 1, :],
                                pT[:, c:c + 1],
                                e3t[:, c, :],
                                start=(ti == 0),
                                stop=(ti == n_tiles - 1),
                                tile_position=(0, 32 * c),
                                skip_group_check=True,
                            )

                for it0 in range(0, n_tiles, 2):
                    enc8s, enc3s = [], []
                    for j in range(2):
                        ti = it0 + j
                        e8 = enc8_pool.tile([128, DC, t], f8e4, tag="enc8")
                        nc.sync.dma_start(out=e8, in_=enc8_ext[b, ti])
                        enc8s.append(e8)
                        e3 = enc3_pool.tile([128, 4, D], f8e3, tag="enc3")
                        nc.sync.dma_start(out=e3, in_=encq3_ext[b, ti])
                        enc3s.append(e3)

                    # h8[p(k), kc, j, s] = tanh((1/16) h_ps + r)
                    h8 = h8_pool.tile([128, KC, 2, t], bf16, tag="h8")
                    for kc in range(KC):
                        h_ps = ps_h.tile([128, 2, t], fp32, tag="h")
                        for c2 in range(2):
                            for j in range(2):
                                nc.tensor.matmul(
                                    h_ps[:, j, :],
                                    w1et8_sb[:, kc, 2 * c2:2 * c2 + 2, :],
                                    enc8s[j][:, 2 * c2:2 * c2 + 2, :],
                                    start=(c2 == 0),
                                    stop=(c2 == 1),
                                    perf_mode=DR,
                                )
                        nc.scalar.activation(
                            out=h8[:, kc, :, :], in_=h_ps, func=AF.Tanh,
                            bias=r_sb[:, kc, b:b + 1], scale=1.0 / W_SCALE,
                        )

                    for j in range(2):
                        ti = it0 + j
                        e_ps = ps_e.tile([1, t], fp32, tag="e")
                        for kc in range(KC):
                            nc.tensor.matmul(
                                e_ps,
                                w2_col[:, kc:kc + 1],
                                h8[:, kc, j, :],
                                start=(kc == 0),
                                stop=(kc == KC - 1),
                            )

                        p_row = p_pool.tile([1, t], bf16, tag="p")
                        nc.scalar.activation(
                            out=p_row, in_=e_ps, func=AF.Exp,
                            accum_out=z_parts[:, ti:ti + 1],
                        )
                        # transpose p to columns: pT[p, c] = p_row[4p + c]
                        pT = pt_pool.tile([128, 4], bf16, tag="pt")
                        nc.scalar.dma_start(out=pT, in_=p_row)
                        pending_ctx.append((pT, enc3s[j], ti))

                    # context MMs of the previous pair (keeps PE fed while the
                    # exp -> pT DMA of this pair is still in flight)
                    if it0 > 0:
                        flush_ctx()
                flush_ctx()

                # ---- batch epilogue ---------------------------------------
                z_red = tiny.tile([1, 1], fp32)
                nc.vector.tensor_reduce(
                    out=z_red, in_=z_parts, axis=mybir.AxisListType.X, op=Alu.add
                )
                zr = tiny.tile([1, 1], fp32)
                nc.vector.reciprocal(out=zr, in_=z_red)

                ctx_sb = tiny.tile([128, t], fp32)
                nc.vector.tensor_copy(out=ctx_sb, in_=ctx_ps)
                ctx_row = ps_e.tile([1, t], fp32, tag="e")
                nc.tensor.matmul(ctx_row, ones4, ctx_sb, start=True, stop=True)
                o_row = tiny.tile([1, t], fp32)
                nc.scalar.mul(o_row, ctx_row, zr)
                nc.gpsimd.dma_start(out=out_ext[b], in_=o_row)

    return nc


# Instruction opcodes whose ISA structs tolerate multi-waits (or that the
# split must not touch). Everything else on this walrus build has a single
# sync-wait slot, so excess waits move onto preceding same-engine NoOps.
_NO_SPLIT = {"EventSemaphore", "Call", "UnconditionalBranch", "RegisterMove"}


def split_multi_waits(nc, limit=1):
    import concourse.mybir as mybir

    ctr = 0
    for fn in nc.m.functions:
        for blk in fn.blocks:
            new = []
            for inst in blk.instructions:
                si = inst.sync_info
                waits = list(si.on_wait) if si is not None and si.on_wait else []
                if inst.opcode not in _NO_SPLIT and len(waits) > limit:
                    extra, keep = waits[:-limit], waits[-limit:]
                    for w in extra:
                        ctr += 1
                        new.append(mybir.InstNoOp(
                            name=f"WSPLIT-{ctr}", engine=inst.engine,
                            sync_info=mybir.SyncInfo(on_wait=[w], on_update=[])))
                    inst.sync_info = mybir.SyncInfo(
                        on_wait=keep,
                        on_update=list(si.on_update) if si.on_update else [])
                new.append(inst)
            blk.instructions = new
    return ctr


def _diffuse_quant(x, qdtype):
    """Error-diffusion rounding of x (f32) to qdtype along the last axis:
    running quantization error is fed into the next element, so partial sums
    of the quantized stream track the exact partial sums within half an ULP.
    """
    out = np.empty(x.shape, dtype=qdtype)
    c = np.zeros(x.shape[:-1], dtype=np.float32)
    for s in range(x.shape[-1]):
        v = x[..., s] + c
        q = v.astype(qdtype)
        out[..., s] = q
        c = v - q.astype(np.float32)
    return out


def _prep_host(hidden_state, encoder_output, W1, b1, w2):
    import ml_dtypes

    bf16 = ml_dtypes.bfloat16
    f8e4 = ml_dtypes.float8_e4m3
    f8e3 = ml_dtypes.float8_e3m4

    n_tiles = S // T
    encT = encoder_output.transpose(0, 2, 1)                 # [B, D, S] f32
    # score copy: e4m3, [b, ti, p, dc, s'] with d = dc*128 + p
    enc8 = np.ascontiguousarray(
        encT.reshape(B, DC, 128, n_tiles, T).transpose(0, 3, 2, 1, 4).astype(f8e4)
    )
    # context copy: e3m4 diffused along s, [b, ti, p, c, d] with s = ti*T + 4p + c
    encq = _diffuse_quant(encT, f8e3).astype(f8e3)           # [B, D, S]
    encq3 = np.ascontiguousarray(
        encq.transpose(0, 2, 1).reshape(B, n_tiles, 128, 4, D)
    )
    w1eT = (W_SCALE * W1[:, :D].T).astype(f8e4)              # [d, k]
    w1et8 = np.ascontiguousarray(
        w1eT.reshape(DC, 128, KC, 128).transpose(2, 1, 0, 3)
    )
    w1hT = W1[:, D:].T.astype(bf16)
    w1ht = np.ascontiguousarray(
        w1hT.reshape(DC, 128, KC, 128).transpose(2, 1, 0, 3)
    )
    w28 = np.ascontiguousarray(w2.reshape(KC, 128).astype(bf16))
    in_maps = []
    for i in range(N_CORES):
        sl = slice(i * B_LOC, (i + 1) * B_LOC)
        in_maps.append({
            "enc8": np.ascontiguousarray(enc8[sl]),
            "encq3": np.ascontiguousarray(encq3[sl]),
            "hid": np.ascontiguousarray(hidden_state[sl].astype(bf16)),
            "w1et8": w1et8,
            "w1ht": w1ht,
            "b1": np.ascontiguousarray(b1.astype(np.float32)),
            "w28": w28,
        })
    return in_maps


def _ensure_ntff_hook():
    """Install the axon NTFF profile hook if the image lacks antenv.axon_hooks."""
    import sys
    import types

    try:
        import antenv.axon_hooks  # noqa: F401
        return
    except ImportError:
        pass
    import antenv

    mod = types.ModuleType("antenv.axon_hooks")
    state = {"hook": None}
    mod.set_axon_ntff_profile_hook = lambda h: state.__setitem__("hook", h)
    mod.get_axon_ntff_profile_hook = lambda: state["hook"]
    sys.modules["antenv.axon_hooks"] = mod
    antenv.axon_hooks = mod
    try:
        from trn_agent_boot.trn_boot import _ntff_profile_via_ctypes

        mod.set_axon_ntff_profile_hook(
            _ntff_profile_via_ctypes("/opt/axon/libaxon_pjrt.so")
        )
    except Exception:
        pass


def run(hidden_state, encoder_output, W1, b1, w2, trace=False):
    from concourse.bass_utils import run_bass_kernel_spmd

    if trace:
        _ensure_ntff_hook()

    nc = build_nc()
    nc.finalize()
    split_multi_waits(nc)
    in_maps = _prep_host(
        np.asarray(hidden_state, dtype=np.float32),
        np.asarray(encoder_output, dtype=np.float32),
        np.asarray(W1, dtype=np.float32),
        np.asarray(b1, dtype=np.float32),
        np.asarray(w2, dtype=np.float32),
    )
    res = run_bass_kernel_spmd(nc, in_maps, core_ids=list(range(N_CORES)), trace=trace)
    out = np.concatenate([res.results[i]["out"] for i in range(N_CORES)], axis=0)
    return out, res


def kernel(**inputs):
    out, _ = run(**inputs)
    return out


# revision 9
# speedup vs baseline: 1.0289x; 1.0289x over previous
"""Bahdanau-attention pooling kernel for TRN2, data-parallel over 8 NeuronCores.

Reference computation (per batch b):
    h   = tanh(enc @ W1enc.T + hid @ W1hid.T + b1)    [S, K]   (K = D = 512)
    e   = h @ w2                                       [S]
    a   = softmax(e)                                   [S]
    ctx = a @ enc                                      [D]

Distribution: batch dim (32) sharded 4-per-core across 8 cores; replicated
weights, no collectives.

v3 design (per core, single pass over the encoder stream):
  - scores: enc ships as e4m3 [d, s] tiles; W1enc pre-scaled x16 and
    quantized e4m3; h-matmuls run in DoubleRow fp8 mode (256-deep
    contraction per pass); the 1/16 rescale is folded into the tanh scale.
    tanh is j-pair fused (one ACT per kc chunk covers both tiles of a pair).
    e-matmul stays bf16 (h8/w2 quantization there costs too much accuracy).
  - context: runs on the PE, not DVE. enc also ships as e3m4 [s, d] tiles
    (natural layout, error-diffusion rounded along s so quantization noise
    cancels in the softmax average), with s interleaved as s = 4p + c so a
    plain [1,512]->[128,4] DMA produces the p-column tiles. Each tile then
    adds 4 rank-1 matmuls (p-col x enc-rows), column-tiled to PSUM partitions
    {0,32,64,96}, accumulated across the whole batch in one PSUM bank.
  - exp's accum_out lands z in per-tile slots, reduced once per batch.
  - softmax is max-free: |e| <= ||w2||_1 ~ 11, exp never overflows fp32.
  - ctx matmuls are emitted one pair late (software pipelining) so the PE
    never head-of-line blocks on the exp -> pT DMA chain.
"""

import numpy as np

B, S, D = 32, 4096, 512
N_CORES = 8
B_LOC = B // N_CORES
T = 512          # s-tile size
KC = D // 128    # 4 k-chunks
DC = D // 128    # 4 d-chunks
W_SCALE = 16.0   # host pre-scale on W1enc before e4m3 quantization


def build_nc(b_loc=B_LOC, s_len=S, t=T):
    import concourse.bass as bass
    import concourse.mybir as mybir
    import concourse.tile as tile

    fp32 = mybir.dt.float32
    f32r = mybir.dt.float32r
    bf16 = mybir.dt.bfloat16
    f8e4 = mybir.dt.float8e4
    f8e3 = mybir.dt.float8e3
    AF = mybir.ActivationFunctionType
    Alu = mybir.AluOpType
    DR = mybir.MatmulPerfMode.DoubleRow

    nc = bass.Bass()

    n_tiles = s_len // t

    enc8_ext = nc.declare_dram_parameter(
        "enc8", [b_loc, n_tiles, 128, DC, t], f8e4, isOutput=False)
    encq3_ext = nc.declare_dram_parameter(
        "encq3", [b_loc, n_tiles, 128, 4, D], f8e3, isOutput=False)
    hid_ext = nc.declare_dram_parameter("hid", [b_loc, D], bf16, isOutput=False)
    w1et8_ext = nc.declare_dram_parameter(
        "w1et8", [KC, 128, DC, 128], f8e4, isOutput=False)
    w1ht_ext = nc.declare_dram_parameter(
        "w1ht", [KC, 128, DC, 128], bf16, isOutput=False)
    b1_ext = nc.declare_dram_parameter("b1", [D], fp32, isOutput=False)
    w28_ext = nc.declare_dram_parameter("w28", [KC, 128], bf16, isOutput=False)
    out_ext = nc.declare_dram_parameter("out", [b_loc, D], fp32, isOutput=True)

    with tile.TileContext(nc) as tc:
        with (
            tc.tile_pool(name="singles", bufs=1) as singles,
            tc.tile_pool(name="enc8_pool", bufs=6) as enc8_pool,
            tc.tile_pool(name="enc3_pool", bufs=6) as enc3_pool,
            tc.tile_pool(name="h8_pool", bufs=3) as h8_pool,
            tc.tile_pool(name="p_pool", bufs=6) as p_pool,
            tc.tile_pool(name="pt_pool", bufs=6) as pt_pool,
            tc.tile_pool(name="tiny", bufs=4) as tiny,
            tc.tile_pool(name="ps_h", bufs=2, space=bass.MemorySpace.PSUM) as ps_h,
            tc.tile_pool(name="ps_e", bufs=2, space=bass.MemorySpace.PSUM) as ps_e,
            tc.tile_pool(name="ps_c", bufs=1, space=bass.MemorySpace.PSUM) as ps_c,
        ):
            # ---- persistent tiles (small gpsimd loads + per-kc weight DMAs) --
            hid_cols = singles.tile([128, b_loc, DC], bf16)  # [p(d), b, d-chunk]
            nc.gpsimd.dma_start(
                out=hid_cols, in_=hid_ext.rearrange("b (c p) -> p b c", p=128)
            )
            b1_col = singles.tile([128, KC], fp32)
            nc.gpsimd.dma_start(out=b1_col, in_=b1_ext.rearrange("(c p) -> p c", p=128))
            w2_col = singles.tile([128, KC], bf16)
            nc.gpsimd.dma_start(out=w2_col, in_=w28_ext.rearrange("c p -> p c"))

            w1et8_sb = singles.tile([128, KC, DC, 128], f8e4)
            w1ht_sb = singles.tile([128, KC, DC, 128], bf16)
            for kc in range(KC):
                nc.scalar.dma_start(out=w1et8_sb[:, kc, :, :], in_=w1et8_ext[kc])
                nc.scalar.dma_start(out=w1ht_sb[:, kc, :, :], in_=w1ht_ext[kc])

            # mask column: 1.0 at partitions {0,32,64,96} picks the 4 ctx rows
            ones4 = singles.tile([128, 1], fp32)
            nc.vector.memset(ones4, 0.0)
            for c in range(4):
                nc.vector.memset(ones4[32 * c:32 * c + 1, :], 1.0)

            r_sb = singles.tile([128, KC, b_loc], fp32)   # [p(k), k-chunk, b]
            z_parts = singles.tile([1, n_tiles], fp32)

            ctx_ps = ps_c.tile([128, t], fp32, tag="ctx")
            nc.vector.memset(ctx_ps, 0.0)

            # ---- r = W1hid @ hid + b1  (bf16, all batches at once) ---------
            for kc in range(KC):
                r_ps = ps_e.tile([128, b_loc], fp32, tag="e")
                for dc in range(DC):
                    nc.tensor.matmul(
                        r_ps,
                        w1ht_sb[:, kc, dc, :],
                        hid_cols[:, :, dc],
                        start=(dc == 0),
                        stop=(dc == DC - 1),
                    )
                nc.vector.tensor_scalar_add(
                    out=r_sb[:, kc, :], in0=r_ps, scalar1=b1_col[:, kc:kc + 1]
                )

            # ---- main loop ------------------------------------------------
            for b in range(b_loc):
                pending_ctx = []  # (pT, enc3_tile, ti) emitted one pair late

                def flush_ctx():
                    while pending_ctx:
                        pT, e3t, ti = pending_ctx.pop(0)
                        for c in range(4):
                            nc.tensor.matmul(
                                ctx_ps[32 * c:32 * c + 1, :],
                                pT[:, c:c + 1],
                                e3t[:, c, :],
                                start=(ti == 0),
                                stop=(ti == n_tiles - 1),
                                tile_position=(0, 32 * c),
                                skip_group_check=True,
                            )

                for it0 in range(0, n_tiles, 2):
                    enc8s, enc3s = [], []
                    for j in range(2):
                        ti = it0 + j
                        e8 = enc8_pool.tile([128, DC, t], f8e4, tag="enc8")
                        nc.sync.dma_start(out=e8, in_=enc8_ext[b, ti])
                        enc8s.append(e8)
                        e3 = enc3_pool.tile([128, 4, D], f8e3, tag="enc3")
                        nc.sync.dma_start(out=e3, in_=encq3_ext[b, ti])
                        enc3s.append(e3)

                    # h8[p(k), kc, j, s] = tanh((1/16) h_ps + r)
                    h8 = h8_pool.tile([128, KC, 2, t], bf16, tag="h8")
                    for kc in range(KC):
                        h_ps = ps_h.tile([128, 2, t], fp32, tag="h")
                        for c2 in range(2):
                            for j in range(2):
                                nc.tensor.matmul(
                                    h_ps[:, j, :],
                                    w1et8_sb[:, kc, 2 * c2:2 * c2 + 2, :],
                                    enc8s[j][:, 2 * c2:2 * c2 + 2, :],
                                    start=(c2 == 0),
                                    stop=(c2 == 1),
                                    perf_mode=DR,
                                )
                        nc.scalar.activation(
                            out=h8[:, kc, :, :], in_=h_ps, func=AF.Tanh,
                            bias=r_sb[:, kc, b:b + 1], scale=1.0 / W_SCALE,
                        )

                    for j in range(2):
                        ti = it0 + j
                        e_ps = ps_e.tile([1, t], fp32, tag="e")
                        for kc in range(KC):
                            nc.tensor.matmul(
                                e_ps,
                                w2_col[:, kc:kc + 1],
                                h8[:, kc, j, :],
                                start=(kc == 0),
                                stop=(kc == KC - 1),
                            )

                        p_row = p_pool.tile([1, t], bf16, tag="p")
                        nc.scalar.activation(
                            out=p_row, in_=e_ps, func=AF.Exp,
                            accum_out=z_parts[:, ti:ti + 1],
                        )
                        # transpose p to columns: pT[p, c] = p_row[4p + c]
                        pT = pt_pool.tile([128, 4], bf16, tag="pt")
                        nc.scalar.dma_start(out=pT, in_=p_row)
                        pending_ctx.append((pT, enc3s[j], ti))

                    # context MMs of the previous pair (keeps PE fed while the
                    # exp -> pT DMA of this pair is still in flight)
                    if it0 > 0:
                        flush_ctx()
                flush_ctx()

                # ---- batch epilogue ---------------------------------------
                z_red = tiny.tile([1, 1], fp32)
                nc.vector.tensor_reduce(
                    out=z_red, in_=z_parts, axis=mybir.AxisListType.X, op=Alu.add
                )
                zr = tiny.tile([1, 1], fp32)
                nc.vector.reciprocal(out=zr, in_=z_red)

                ctx_sb = tiny.tile([128, t], fp32)
                nc.vector.tensor_copy(out=ctx_sb, in_=ctx_ps)
                ctx_row = ps_e.tile([1, t], fp32, tag="e")
                nc.tensor.matmul(ctx_row, ones4, ctx_sb, start=True, stop=True)
                o_row = tiny.tile([1, t], fp32)
                nc.scalar.mul(o_row, ctx_row, zr)
                nc.gpsimd.dma_start(out=out_ext[b], in_=o_row)

    return nc


# Instruction opcodes whose ISA structs tolerate multi-waits (or that the
# split must not touch). Everything else on this walrus build has a single
# sync-wait slot, so excess waits move onto preceding same-engine NoOps.
_NO_SPLIT = {"EventSemaphore", "Call", "UnconditionalBranch", "RegisterMove"}


def split_multi_waits(nc, limit=1):
    import concourse.mybir as mybir

    ctr = 0
    for fn in nc.m.functions:
        for blk in fn.blocks:
            new = []
            for inst in blk.instructions:
                si = inst.sync_info
                waits = list(si.on_wait) if si is not None and si.on_wait else []
                if inst.opcode not in _NO_SPLIT and len(waits) > limit:
                    extra, keep = waits[:-limit], waits[-limit:]
                    for w in extra:
                        ctr += 1
                        new.append(mybir.InstNoOp(
                            name=f"WSPLIT-{ctr}", engine=inst.engine,
                            sync_info=mybir.SyncInfo(on_wait=[w], on_update=[])))
                    inst.sync_info = mybir.SyncInfo(
                        on_wait=keep,
                        on_update=list(si.on_update) if si.on_update else [])
                new.append(inst)
            blk.instructions = new
    return ctr


def _diffuse_quant(x, qdtype):
    """Error-diffusion rounding of x (f32) to qdtype along the last axis:
    running quantization error is fed into the next element, so partial sums
    of the quantized stream track the exact partial sums within half an ULP.
    """
    out = np.empty(x.shape, dtype=qdtype)
    c = np.zeros(x.shape[:-1], dtype=np.float32)
    for s in range(x.shape[-1]):
        v = x[..., s] + c
        q = v.astype(qdtype)
        out[..., s] = q
        c = v - q.astype(np.float32)
    return out


def _prep_host(hidden_state, encoder_output, W1, b1, w2):
    import ml_dtypes

    bf16 = ml_dtypes.bfloat16
    f8e4 = ml_dtypes.float8_e4m3
    f8e3 = ml_dtypes.float8_e3m4

    n_tiles = S // T
    encT = encoder_output.transpose(0, 2, 1)                 # [B, D, S] f32
    # score copy: e4m3, [b, ti, p, dc, s'] with d = dc*128 + p
    enc8 = np.ascontiguousarray(
        encT.reshape(B, DC, 128, n_tiles, T).transpose(0, 3, 2, 1, 4).astype(f8e4)
    )
    # context copy: e3m4 diffused along s, [b, ti, p, c, d] with s = ti*T + 4p + c
    encq = _diffuse_quant(encT, f8e3).astype(f8e3)           # [B, D, S]
    encq3 = np.ascontiguousarray(
        encq.transpose(0, 2, 1).reshape(B, n_tiles, 128, 4, D)
    )
    w1eT = (W_SCALE * W1[:, :D].T).astype(f8e4)              # [d, k]
    w1et8 = np.ascontiguousarray(
        w1eT.reshape(DC, 128, KC, 128).transpose(2, 1, 0, 3)
    )
    w1hT = W1[:, D:].T.astype(bf16)
    w1ht = np.ascontiguousarray(
        w1hT.reshape(DC, 128, KC, 128).transpose(2, 1, 0, 3)
    )
    w28 = np.ascontiguousarray(w2.reshape(KC, 128).astype(bf16))
    in_maps = []
    for i in range(N_CORES):
        sl = slice(i * B_LOC, (i + 1) * B_LOC)
        in_maps.append({
            "enc8": np.ascontiguousarray(enc8[sl]),
            "encq3": np.ascontiguousarray(encq3[sl]),
            "hid": np.ascontiguousarray(hidden_state[sl].astype(bf16)),
            "w1et8": w1et8,
            "w1ht": w1ht,
            "b1": np.ascontiguousarray(b1.astype(np.float32)),
            "w28": w28,
        })
    return in_maps


def _ensure_ntff_hook():
    """Install the axon NTFF profile hook if the image lacks antenv.axon_hooks."""
    import sys
    import types

    try:
        import antenv.axon_hooks  # noqa: F401
        return
    except ImportError:
        pass
    import antenv

    mod = types.ModuleType("antenv.axon_hooks")
    state = {"hook": None}
    mod.set_axon_ntff_profile_hook = lambda h: state.__setitem__("hook", h)
    mod.get_axon_ntff_profile_hook = lambda: state["hook"]
    sys.modules["antenv.axon_hooks"] = mod
    antenv.axon_hooks = mod
    try:
        from trn_agent_boot.trn_boot import _ntff_profile_via_ctypes

        mod.set_axon_ntff_profile_hook(
            _ntff_profile_via_ctypes("/opt/axon/libaxon_pjrt.so")
        )
    except Exception:
        pass


def run(hidden_state, encoder_output, W1, b1, w2, trace=False):
    from concourse.bass_utils import run_bass_kernel_spmd

    if trace:
        _ensure_ntff_hook()

    nc = build_nc()
    nc.finalize()
    split_multi_waits(nc)
    in_maps = _prep_host(
        np.asarray(hidden_state, dtype=np.float32),
        np.asarray(encoder_output, dtype=np.float32),
        np.asarray(W1, dtype=np.float32),
        np.asarray(b1, dtype=np.float32),
        np.asarray(w2, dtype=np.float32),
    )
    res = run_bass_kernel_spmd(nc, in_maps, core_ids=list(range(N_CORES)), trace=trace)
    out = np.concatenate([res.results[i]["out"] for i in range(N_CORES)], axis=0)
    return out, res


def kernel(**inputs):
    out, _ = run(**inputs)
    return out


# revision 11
# speedup vs baseline: 1.0693x; 1.0393x over previous
"""Bahdanau-attention pooling kernel for TRN2, data-parallel over 8 NeuronCores.

Reference computation (per batch b):
    h   = tanh(enc @ W1enc.T + hid @ W1hid.T + b1)    [S, K]   (K = D = 512)
    e   = h @ w2                                       [S]
    a   = softmax(e)                                   [S]
    ctx = a @ enc                                      [D]

Distribution: batch dim (32) sharded 4-per-core across 8 cores; replicated
weights, no collectives.

v3 design (per core, single pass over the encoder stream):
  - scores: enc ships as e4m3 [d, s] tiles; W1enc pre-scaled x16 and
    quantized e4m3; h-matmuls run in DoubleRow fp8 mode (256-deep
    contraction per pass); the 1/16 rescale is folded into the tanh scale.
    tanh is j-pair fused (one ACT per kc chunk covers both tiles of a pair).
    e-matmul stays bf16 (h8/w2 quantization there costs too much accuracy).
  - context: runs on the PE, not DVE. enc also ships as e3m4 [s, d] tiles
    (natural layout, error-diffusion rounded along s so quantization noise
    cancels in the softmax average), with s interleaved as s = 4p + c so a
    plain [1,512]->[128,4] DMA produces the p-column tiles. Each tile then
    adds 4 rank-1 matmuls (p-col x enc-rows), column-tiled to PSUM partitions
    {0,32,64,96}, accumulated across the whole batch in one PSUM bank.
  - exp's accum_out lands z in per-tile slots, reduced once per batch.
  - softmax is max-free: |e| <= ||w2||_1 ~ 11, exp never overflows fp32.
  - ctx matmuls are emitted one pair late (software pipelining) so the PE
    never head-of-line blocks on the exp -> pT DMA chain.
"""

import numpy as np

B, S, D = 32, 4096, 512
N_CORES = 8
B_LOC = B // N_CORES
T = 512          # s-tile size
KC = D // 128    # 4 k-chunks
DC = D // 128    # 4 d-chunks
W_SCALE = 16.0   # host pre-scale on W1enc before e4m3 quantization


def build_nc(b_loc=B_LOC, s_len=S, t=T):
    import concourse.bass as bass
    import concourse.mybir as mybir
    import concourse.tile as tile

    fp32 = mybir.dt.float32
    f32r = mybir.dt.float32r
    bf16 = mybir.dt.bfloat16
    f8e4 = mybir.dt.float8e4
    f8e3 = mybir.dt.float8e3
    AF = mybir.ActivationFunctionType
    Alu = mybir.AluOpType
    DR = mybir.MatmulPerfMode.DoubleRow

    nc = bass.Bass()

    n_tiles = s_len // t

    enc8_ext = nc.declare_dram_parameter(
        "enc8", [b_loc, n_tiles, 128, DC, t], f8e4, isOutput=False)
    encq3_ext = nc.declare_dram_parameter(
        "encq3", [b_loc, n_tiles, 128, 4, D], f8e3, isOutput=False)
    hid_ext = nc.declare_dram_parameter("hid", [b_loc, D], bf16, isOutput=False)
    w1et8_ext = nc.declare_dram_parameter(
        "w1et8", [KC, 128, DC, 128], f8e4, isOutput=False)
    w1ht_ext = nc.declare_dram_parameter(
        "w1ht", [KC, 128, DC, 128], bf16, isOutput=False)
    b1_ext = nc.declare_dram_parameter("b1", [D], fp32, isOutput=False)
    w28_ext = nc.declare_dram_parameter("w28", [KC, 128], bf16, isOutput=False)
    out_ext = nc.declare_dram_parameter("out", [b_loc, D], fp32, isOutput=True)

    with tile.TileContext(nc) as tc:
        with (
            tc.tile_pool(name="singles", bufs=1) as singles,
            tc.tile_pool(name="enc8_pool", bufs=6) as enc8_pool,
            tc.tile_pool(name="enc3_pool", bufs=6) as enc3_pool,
            tc.tile_pool(name="h8_pool", bufs=3) as h8_pool,
            tc.tile_pool(name="p_pool", bufs=6) as p_pool,
            tc.tile_pool(name="pt_pool", bufs=6) as pt_pool,
            tc.tile_pool(name="tiny", bufs=4) as tiny,
            tc.tile_pool(name="ps_h", bufs=5, space=bass.MemorySpace.PSUM) as ps_h,
            tc.tile_pool(name="ps_e", bufs=2, space=bass.MemorySpace.PSUM) as ps_e,
            tc.tile_pool(name="ps_c", bufs=1, space=bass.MemorySpace.PSUM) as ps_c,
        ):
            # ---- persistent tiles (small gpsimd loads + per-kc weight DMAs) --
            hid_cols = singles.tile([128, b_loc, DC], bf16)  # [p(d), b, d-chunk]
            nc.scalar.dma_start(
                out=hid_cols, in_=hid_ext.rearrange("b (c p) -> p b c", p=128)
            )
            b1_col = singles.tile([128, KC], fp32)
            nc.scalar.dma_start(out=b1_col, in_=b1_ext.rearrange("(c p) -> p c", p=128))
            w2_col = singles.tile([128, KC], bf16)
            nc.scalar.dma_start(out=w2_col, in_=w28_ext.rearrange("c p -> p c"))

            w1et8_sb = singles.tile([128, KC, DC, 128], f8e4)
            w1ht_sb = singles.tile([128, KC, DC, 128], bf16)
            for kc in range(KC):
                nc.scalar.dma_start(out=w1et8_sb[:, kc, :, :], in_=w1et8_ext[kc])
                nc.scalar.dma_start(out=w1ht_sb[:, kc, :, :], in_=w1ht_ext[kc])

            # mask column: 1.0 at partitions {0,32,64,96} picks the 4 ctx rows
            ones4 = singles.tile([128, 1], fp32)
            nc.vector.memset(ones4, 0.0)
            for c in range(4):
                nc.vector.memset(ones4[32 * c:32 * c + 1, :], 1.0)

            r_sb = singles.tile([128, KC, b_loc], fp32)   # [p(k), k-chunk, b]
            z_parts = singles.tile([1, n_tiles], fp32)

            ctx_ps = ps_c.tile([128, t], fp32, tag="ctx")
            nc.vector.memset(ctx_ps, 0.0)

            # ---- r = W1hid @ hid + b1  (bf16, all batches at once) ---------
            for kc in range(KC):
                r_ps = ps_e.tile([128, b_loc], fp32, tag="e")
                for dc in range(DC):
                    nc.tensor.matmul(
                        r_ps,
                        w1ht_sb[:, kc, dc, :],
                        hid_cols[:, :, dc],
                        start=(dc == 0),
                        stop=(dc == DC - 1),
                    )
                nc.vector.tensor_scalar_add(
                    out=r_sb[:, kc, :], in0=r_ps, scalar1=b1_col[:, kc:kc + 1]
                )

            # ---- main loop ------------------------------------------------
            for b in range(b_loc):
                pending_ctx = []  # (pT, enc3_tile, ti) emitted one pair late

                def flush_ctx():
                    while pending_ctx:
                        pT, e3t, ti = pending_ctx.pop(0)
                        for c in range(4):
                            nc.tensor.matmul(
                                ctx_ps[32 * c:32 * c + 1, :],
                                pT[:, c:c + 1],
                                e3t[:, c, :],
                                start=(ti == 0),
                                stop=(ti == n_tiles - 1),
                                tile_position=(0, 32 * c),
                                skip_group_check=True,
                            )

                for it0 in range(0, n_tiles, 2):
                    enc8s, enc3s = [], []
                    for j in range(2):
                        ti = it0 + j
                        e8 = enc8_pool.tile([128, DC, t], f8e4, tag="enc8")
                        nc.sync.dma_start(out=e8, in_=enc8_ext[b, ti])
                        enc8s.append(e8)
                        e3 = enc3_pool.tile([128, 4, D], f8e3, tag="enc3")
                        nc.sync.dma_start(out=e3, in_=encq3_ext[b, ti])
                        enc3s.append(e3)

                    # h8[p(k), kc, j, s] = tanh((1/16) h_ps + r)
                    h8 = h8_pool.tile([128, KC, 2, t], bf16, tag="h8")
                    for kc in range(KC):
                        h_pss = [ps_h.tile([128, t], fp32, tag="h", name=f"hps{j}") for j in range(2)]
                        for c2 in range(2):
                            for j in range(2):
                                nc.tensor.matmul(
                                    h_pss[j],
                                    w1et8_sb[:, kc, 2 * c2:2 * c2 + 2, :],
                                    enc8s[j][:, 2 * c2:2 * c2 + 2, :],
                                    start=(c2 == 0),
                                    stop=(c2 == 1),
                                    perf_mode=DR,
                                )
                        for j in range(2):
                            nc.scalar.activation(
                                out=h8[:, kc, j, :], in_=h_pss[j], func=AF.Tanh,
                                bias=r_sb[:, kc, b:b + 1], scale=1.0 / W_SCALE,
                            )

                    for j in range(2):
                        ti = it0 + j
                        e_ps = ps_e.tile([1, t], fp32, tag="e")
                        for kc in range(KC):
                            nc.tensor.matmul(
                                e_ps,
                                w2_col[:, kc:kc + 1],
                                h8[:, kc, j, :],
                                start=(kc == 0),
                                stop=(kc == KC - 1),
                            )

                        p_row = p_pool.tile([1, t], bf16, tag="p")
                        nc.scalar.activation(out=p_row, in_=e_ps, func=AF.Exp)
                        nc.vector.tensor_reduce(
                            out=z_parts[:, ti:ti + 1], in_=p_row,
                            axis=mybir.AxisListType.X, op=Alu.add,
                        )
                        # transpose p to columns: pT[p, c] = p_row[4p + c]
                        pT = pt_pool.tile([128, 4], bf16, tag="pt")
                        nc.scalar.dma_start(out=pT, in_=p_row)
                        pending_ctx.append((pT, enc3s[j], ti))

                    # context MMs of the previous pair (keeps PE fed while the
                    # exp -> pT DMA of this pair is still in flight)
                    if it0 > 0:
                        flush_ctx()
                flush_ctx()

                # ---- batch epilogue ---------------------------------------
                z_red = tiny.tile([1, 1], fp32)
                nc.vector.tensor_reduce(
                    out=z_red, in_=z_parts, axis=mybir.AxisListType.X, op=Alu.add
                )
                zr = tiny.tile([1, 1], fp32)
                nc.vector.reciprocal(out=zr, in_=z_red)

                ctx_sb = tiny.tile([128, t], fp32)
                nc.vector.tensor_copy(out=ctx_sb, in_=ctx_ps)
                ctx_row = ps_e.tile([1, t], fp32, tag="e")
                nc.tensor.matmul(ctx_row, ones4, ctx_sb, start=True, stop=True)
                o_row = tiny.tile([1, t], fp32)
                nc.scalar.mul(o_row, ctx_row, zr)
                nc.gpsimd.dma_start(out=out_ext[b], in_=o_row)

    return nc


# Instruction opcodes whose ISA structs tolerate multi-waits (or that the
# split must not touch). Everything else on this walrus build has a single
# sync-wait slot, so excess waits move onto preceding same-engine NoOps.
_NO_SPLIT = {"EventSemaphore", "Call", "UnconditionalBranch", "RegisterMove"}


def split_multi_waits(nc, limit=1):
    import concourse.mybir as mybir

    ctr = 0
    for fn in nc.m.functions:
        for blk in fn.blocks:
            new = []
            for inst in blk.instructions:
                si = inst.sync_info
                waits = list(si.on_wait) if si is not None and si.on_wait else []
                if inst.opcode not in _NO_SPLIT and len(waits) > limit:
                    extra, keep = waits[:-limit], waits[-limit:]
                    for w in extra:
                        ctr += 1
                        new.append(mybir.InstNoOp(
                            name=f"WSPLIT-{ctr}", engine=inst.engine,
                            sync_info=mybir.SyncInfo(on_wait=[w], on_update=[])))
                    inst.sync_info = mybir.SyncInfo(
                        on_wait=keep,
                        on_update=list(si.on_update) if si.on_update else [])
                new.append(inst)
            blk.instructions = new
    return ctr


def _diffuse_quant(x, qdtype):
    """Error-diffusion rounding of x (f32) to qdtype along the last axis:
    running quantization error is fed into the next element, so partial sums
    of the quantized stream track the exact partial sums within half an ULP.
    """
    out = np.empty(x.shape, dtype=qdtype)
    c = np.zeros(x.shape[:-1], dtype=np.float32)
    for s in range(x.shape[-1]):
        v = x[..., s] + c
        q = v.astype(qdtype)
        out[..., s] = q
        c = v - q.astype(np.float32)
    return out


def _prep_host(hidden_state, encoder_output, W1, b1, w2):
    import ml_dtypes

    bf16 = ml_dtypes.bfloat16
    f8e4 = ml_dtypes.float8_e4m3
    f8e3 = ml_dtypes.float8_e3m4

    n_tiles = S // T
    encT = encoder_output.transpose(0, 2, 1)                 # [B, D, S] f32
    # score copy: e4m3, [b, ti, p, dc, s'] with d = dc*128 + p
    enc8 = np.ascontiguousarray(
        encT.reshape(B, DC, 128, n_tiles, T).transpose(0, 3, 2, 1, 4).astype(f8e4)
    )
    # context copy: e3m4 diffused along s, [b, ti, p, c, d] with s = ti*T + 4p + c
    encq = _diffuse_quant(encT, f8e3).astype(f8e3)           # [B, D, S]
    encq3 = np.ascontiguousarray(
        encq.transpose(0, 2, 1).reshape(B, n_tiles, 128, 4, D)
    )
    w1eT = (W_SCALE * W1[:, :D].T).astype(f8e4)              # [d, k]
    w1et8 = np.ascontiguousarray(
        w1eT.reshape(DC, 128, KC, 128).transpose(2, 1, 0, 3)
    )
    w1hT = W1[:, D:].T.astype(bf16)
    w1ht = np.ascontiguousarray(
        w1hT.reshape(DC, 128, KC, 128).transpose(2, 1, 0, 3)
    )
    w28 = np.ascontiguousarray(w2.reshape(KC, 128).astype(bf16))
    in_maps = []
    for i in range(N_CORES):
        sl = slice(i * B_LOC, (i + 1) * B_LOC)
        in_maps.append({
            "enc8": np.ascontiguousarray(enc8[sl]),
            "encq3": np.ascontiguousarray(encq3[sl]),
            "hid": np.ascontiguousarray(hidden_state[sl].astype(bf16)),
            "w1et8": w1et8,
            "w1ht": w1ht,
            "b1": np.ascontiguousarray(b1.astype(np.float32)),
            "w28": w28,
        })
    return in_maps


def _ensure_ntff_hook():
    """Install the axon NTFF profile hook if the image lacks antenv.axon_hooks."""
    import sys
    import types

    try:
        import antenv.axon_hooks  # noqa: F401
        return
    except ImportError:
        pass
    import antenv

    mod = types.ModuleType("antenv.axon_hooks")
    state = {"hook": None}
    mod.set_axon_ntff_profile_hook = lambda h: state.__setitem__("hook", h)
    mod.get_axon_ntff_profile_hook = lambda: state["hook"]
    sys.modules["antenv.axon_hooks"] = mod
    antenv.axon_hooks = mod
    try:
        from trn_agent_boot.trn_boot import _ntff_profile_via_ctypes

        mod.set_axon_ntff_profile_hook(
            _ntff_profile_via_ctypes("/opt/axon/libaxon_pjrt.so")
        )
    except Exception:
        pass


def run(hidden_state, encoder_output, W1, b1, w2, trace=False):
    from concourse.bass_utils import run_bass_kernel_spmd

    if trace:
        _ensure_ntff_hook()

    nc = build_nc()
    nc.finalize()
    split_multi_waits(nc)
    in_maps = _prep_host(
        np.asarray(hidden_state, dtype=np.float32),
        np.asarray(encoder_output, dtype=np.float32),
        np.asarray(W1, dtype=np.float32),
        np.asarray(b1, dtype=np.float32),
        np.asarray(w2, dtype=np.float32),
    )
    res = run_bass_kernel_spmd(nc, in_maps, core_ids=list(range(N_CORES)), trace=trace)
    out = np.concatenate([res.results[i]["out"] for i in range(N_CORES)], axis=0)
    return out, res


def kernel(**inputs):
    out, _ = run(**inputs)
    return out
